# revision 1
# baseline (speedup 1.0000x reference)
"""EdgeConv-style GNN message passing kernel for Trainium2 (Bass/Tile).

Computes, for each edge e = (s, d):
    proj = x @ w1.T + b1                      # [N, H]  (node projection)
    h_e  = relu(proj[s] + proj[d])            # [E, H]
    out_e = [h_e | edge_attr_e | edge_f_e] @ w2.T + b2   # [E, O]

Sharding: edges are split evenly across 8 NeuronCores; x and the small
linear weights are replicated.  Each core computes the full proj table
locally, stores it in a DRAM scratch buffer, then gathers the two
endpoint rows per edge with the batched SWDGE gather (InstDMAGatherAnt).

That gather takes int16 indices (max 32767), so nodes are addressed with
a stride-4 trick: gather pass r reads rows at base offset r rows with row
stride 4 rows (1024B), index = node>>2 (<= 25087).  Edges are bucketed on
the host by (src&3, dst&3) into 16 blocks of 512 slots per 8192-slot
macro; the host permutes edge_attr/edge_f into that slot order and
inverse-permutes the output rows during unshard.  Only the low 2 bits of
the node ids drive the bucketing, so the gather stays random-access.

g_all row layout per macro, in units of 128 rows (gathered 16384 rows):
    [sr0(16u) | ds0(16u) | sr1 | ds1 | sr2 | ds2 | sr3 | ds3]
  - gather call r writes units [r*32, (r+1)*32)
  - src rows of block (r,s) at unit  r*32 + s*4      (4 units = 512 rows)
  - dst rows of block (r,s) at unit  s*32 + 16 + r*4
  - hs (edge slot) unit of block (r,s) = r*16 + s*4
"""

import math

import numpy as np

import concourse.bacc as bacc
import concourse.bass as bass
import concourse.mybir as mybir
from concourse import library_config
from concourse.bass_utils import run_bass_kernel_spmd
from concourse.masks import make_identity
from concourse.tile import TileContext, add_dep_helper

F32 = mybir.dt.float32
I16 = mybir.dt.int16
RELU = mybir.ActivationFunctionType.Relu

N_CORES = 8
NF = 64   # node feature dim (lin1 input)
NH = 64   # hidden dim (lin1 output)
EA = 16   # edge_attr dim
EF = 16   # edge_f dim
CF = NH + EA + EF  # concat feature dim = 96
OD = 64   # output dim

NODE_MACRO = 1024         # nodes per phase-1 macro tile (8 blocks of 128)
BLK = 512                 # edges per (r,s) bucket block
MACRO = 16 * BLK          # 8192 edge slots per phase-2 macro
N_GROUPS = MACRO // 512   # 16 groups of 512 edge slots per macro

TRACE = False
LAST_RESULTS = None


def _build_nc(
    n_pad: int, nm_edge: int, b1_nz: bool, b2_nz: bool, p2_only: bool = False
) -> bass.Bass:
    assert n_pad % NODE_MACRO == 0
    nm_node = n_pad // NODE_MACRO
    e_slots = nm_edge * MACRO

    nc = bacc.Bacc()
    x = nc.declare_dram_parameter("x", [n_pad, NF], F32, isOutput=False)
    w1t = nc.declare_dram_parameter("w1t", [128, NH], F32, isOutput=False)
    w2t = nc.declare_dram_parameter("w2t", [CF, OD], F32, isOutput=False)
    # per macro: 4 gather calls x 4096 int16 idx, each wrapped [128, 256]
    idx = nc.declare_dram_parameter("idx", [nm_edge, 128, 1024], I16, isOutput=False)
    ea = nc.declare_dram_parameter("ea", [e_slots, EA], F32, isOutput=False)
    ef = nc.declare_dram_parameter("ef", [e_slots, EF], F32, isOutput=False)
    if b1_nz:
        b1r = nc.declare_dram_parameter("b1r", [1, 512], F32, isOutput=False)
    if b2_nz:
        b2r = nc.declare_dram_parameter("b2r", [128, 256], F32, isOutput=False)
    out = nc.declare_dram_parameter("out", [e_slots, OD], F32, isOutput=True)
    if p2_only:
        proj = nc.declare_dram_parameter("proj", [n_pad, NH], F32, isOutput=False)
    else:
        proj = nc.dram_tensor("proj", [n_pad, NH], F32)
    proj4 = proj[:, :].rearrange("(q r) f -> q r f", r=4)

    with TileContext(nc) as tc:
        with tc.tile_pool(name="const", bufs=1) as cpool:
            libload = nc.gpsimd.load_library(library_config.mlp)
            ident = cpool.tile([128, 128], F32)
            make_identity(nc, ident[:])
            w1t_sb = cpool.tile([128, NH], F32)
            nc.sync.dma_start(out=w1t_sb[:], in_=w1t[:])
            w2t_sb = cpool.tile([CF, OD], F32)
            nc.sync.dma_start(out=w2t_sb[:], in_=w2t[:])
            if b1_nz:
                b1r_sb = cpool.tile([1, 512], F32)
                nc.sync.dma_start(out=b1r_sb[:], in_=b1r[:])
                ones_sb = cpool.tile([1, 128], F32)
                nc.gpsimd.memset(ones_sb[:], 1.0)
            if b2_nz:
                b2r_sb = cpool.tile([128, 256], F32)
                nc.sync.dma_start(out=b2r_sb[:], in_=b2r[:])

            # ---------------- phase 1: proj = x @ w1.T (+ b1) ----------------
            proj_stores = []
            with (
                tc.tile_pool(name="p1", bufs=3) as p1pool,
                tc.tile_pool(name="p1psA", bufs=2, space="PSUM") as ps_xt_pool,
                tc.tile_pool(name="p1psB", bufs=2, space="PSUM") as ps_pr_pool,
            ):
                for m in range(0 if p2_only else nm_node):
                    x_t = p1pool.tile([128, 8, NF], F32, tag="x")
                    nc.sync.dma_start(
                        out=x_t[:],
                        in_=x[m * 1024:(m + 1) * 1024].rearrange(
                            "(t p) f -> p t f", p=128
                        ),
                    )
                    # single-block transposes: everything stays at SBUF/PSUM
                    # partition 0 (partition-offset matmul operands crash HW)
                    xT_sb = p1pool.tile([64, 1024], F32, tag="xT")
                    for half in range(2):
                        ps_xT = ps_xt_pool.tile([64, 512], F32, tag="psxT")
                        for t4 in range(4):
                            t = half * 4 + t4
                            nc.tensor.transpose(
                                out=ps_xT[:, t4 * 128:(t4 + 1) * 128],
                                in_=x_t[:, t, :],
                                identity=ident[:],
                            )
                        nc.vector.tensor_copy(
                            out=xT_sb[:, half * 512:(half + 1) * 512],
                            in_=ps_xT[:],
                        )
                    ps_proj = ps_pr_pool.tile([128, 512], F32, tag="psproj")
                    if b1_nz:
                        nc.tensor.matmul(
                            out=ps_proj[:],
                            lhsT=ones_sb[:1, :],
                            rhs=b1r_sb[:1, :],
                            start=True,
                            stop=False,
                            skip_group_check=True,
                        )
                    for t in range(8):
                        nc.tensor.matmul(
                            out=ps_proj[:, t * 64:(t + 1) * 64],
                            lhsT=xT_sb[:, t * 128:(t + 1) * 128],
                            rhs=w1t_sb[:64, :],
                            start=not b1_nz,
                            stop=(t == 7) if b1_nz else True,
                            skip_group_check=b1_nz,
                        )
                    proj_sb = p1pool.tile([128, 512], F32, tag="proj")
                    nc.scalar.copy(out=proj_sb[:], in_=ps_proj[:])
                    st = nc.scalar.dma_start(
                        out=proj[m * 1024:(m + 1) * 1024].rearrange(
                            "(t p) f -> p t f", p=128
                        ),
                        in_=proj_sb[:].rearrange("p (t f) -> p t f", t=8),
                    )
                    proj_stores.append(st)

            join = None
            if not p2_only:
                join = nc.sync.nop(nofuse=True, hint="proj_done_join")
                for st in proj_stores:
                    add_dep_helper(
                        join.ins, st.ins, reason="join waits on proj store"
                    )

            # ---------------- phase 2: per-edge compute ----------------
            with (
                tc.tile_pool(name="p2idx", bufs=2) as idxpool,
                tc.tile_pool(name="p2g", bufs=2) as gpool,
                tc.tile_pool(name="p2hs", bufs=2) as hspool,
                tc.tile_pool(name="p2af", bufs=2) as afpool,
                tc.tile_pool(name="p2ft", bufs=4) as ftpool,
                tc.tile_pool(name="p2o", bufs=3) as opool,
                tc.tile_pool(name="p2psF", bufs=3, space="PSUM") as psf_pool,
                tc.tile_pool(name="p2psO", bufs=3, space="PSUM") as pso_pool,
            ):
                for m in range(nm_edge):
                    idx_t = idxpool.tile([128, 1024], I16, tag="idx")
                    nc.gpsimd.dma_start(out=idx_t[:], in_=idx[m])
                    g = gpool.tile([128, 128, NH], F32, tag="g")
                    for r in range(4):
                        gi = nc.gpsimd.dma_gather(
                            out_ap=g[:, r * 32:(r + 1) * 32, :],
                            in_ap=proj4[:, r, :],
                            idxs_ap=idx_t[:, r * 256:(r + 1) * 256],
                            num_idxs=4096,
                            num_idxs_reg=4096,
                            elem_size=NH,
                            elem_step=4 * NH,
                            single_packet=False,
                        )
                        add_dep_helper(
                            gi.ins, libload.ins, reason="gather after lib load"
                        )
                        if join is not None:
                            add_dep_helper(
                                gi.ins, join.ins, reason="gather waits on proj"
                            )
                    hs = hspool.tile([128, 64, NH], F32, tag="hs")
                    # per-(r,s)-block adds: each depends on only 2 gather
                    # calls, so they overlap the remaining gathers
                    for r in range(4):
                        for s in range(4):
                            su = r * 32 + s * 4
                            du = s * 32 + 16 + r * 4
                            hu = r * 16 + s * 4
                            nc.vector.tensor_add(
                                out=hs[:, hu:hu + 4, :],
                                in0=g[:, su:su + 4, :],
                                in1=g[:, du:du + 4, :],
                            )
                    asm = afpool.tile([128, 64, CF], F32, tag="asm")
                    nc.scalar.activation(
                        out=asm[:, :, 0:NH], in_=hs[:], func=RELU
                    )
                    base_e = m * MACRO
                    nc.gpsimd.dma_start(
                        out=asm[:, :, NH:NH + EA],
                        in_=ea[base_e:base_e + MACRO].rearrange(
                            "(j p) f -> p j f", p=128
                        ),
                    )
                    nc.gpsimd.dma_start(
                        out=asm[:, :, NH + EA:CF],
                        in_=ef[base_e:base_e + MACRO].rearrange(
                            "(j p) f -> p j f", p=128
                        ),
                    )
                    for grp in range(N_GROUPS):
                        ps_f = psf_pool.tile([CF, 512], F32, tag="psf")
                        for j4 in range(4):
                            j = grp * 4 + j4
                            nc.tensor.transpose(
                                out=ps_f[:, j4 * 128:(j4 + 1) * 128],
                                in_=asm[:, j, :],
                                identity=ident[:],
                            )
                        fT = ftpool.tile([CF, 512], F32, tag="ft")
                        if grp % 2 == 0:
                            nc.vector.tensor_copy(out=fT[:], in_=ps_f[:])
                        else:
                            nc.scalar.copy(out=fT[:], in_=ps_f[:])
                        ps_o = pso_pool.tile([128, 256], F32, tag="pso")
                        for j4 in range(4):
                            nc.tensor.matmul(
                                out=ps_o[:, j4 * 64:(j4 + 1) * 64],
                                lhsT=fT[:, j4 * 128:(j4 + 1) * 128],
                                rhs=w2t_sb[:],
                                start=True,
                                stop=True,
                            )
                        if grp % 8 == 0:
                            o_big = opool.tile([128, 8, 256], F32, tag="o")
                        o_sb = o_big[:, grp % 8, :]
                        if b2_nz:
                            nc.vector.tensor_add(
                                out=o_sb, in0=ps_o[:], in1=b2r_sb[:]
                            )
                        elif grp % 2 == 0:
                            nc.scalar.copy(out=o_sb, in_=ps_o[:])
                        else:
                            nc.vector.tensor_copy(out=o_sb, in_=ps_o[:])
                        if grp % 8 == 7:
                            base = base_e + (grp - 7) * 512
                            nc.sync.dma_start(
                                out=out[base:base + 4096].rearrange(
                                    "(g j p) f -> p (g j) f", p=128, j=4
                                ),
                                in_=o_big[:].rearrange("p g (j f) -> p (g j) f", j=4),
                            )
    nc.compile()
    return nc


def _shard_core(src, dst, nm_edge):
    """Bucket one core's edges by (src&3, dst&3) into the macro/block layout.

    Returns (pos, idx16) where pos[e] is the edge's slot index in
    [0, nm_edge*MACRO) and idx16 is the [nm_edge, 128, 1024] int16 gather
    index tensor.
    """
    n = src.shape[0]
    key = ((src & 3) << 2 | (dst & 3)).astype(np.int8)
    order = np.argsort(key, kind="stable")
    sorted_key = key[order]
    # rank of each sorted element within its bucket
    bstart = np.searchsorted(sorted_key, np.arange(16))
    wb = np.arange(n) - bstart[sorted_key]
    r = (sorted_key >> 2).astype(np.int64)
    s = (sorted_key & 3).astype(np.int64)
    chunk = wb // BLK
    off = wb % BLK
    slot_sorted = chunk * MACRO + (r * 16 + s * 4) * 128 + off
    pos = np.empty(n, dtype=np.int64)
    pos[order] = slot_sorted

    # gather index arrays: SRCV[m, r, s, off] / DSTV[m, s, r, off]
    srcv = np.zeros((nm_edge, 4, 4, BLK), dtype=np.int16)
    dstv = np.zeros((nm_edge, 4, 4, BLK), dtype=np.int16)
    srcq = (src[order] >> 2).astype(np.int16)
    dstq = (dst[order] >> 2).astype(np.int16)
    srcv[chunk, r, s, off] = srcq
    dstv[chunk, s, r, off] = dstq

    idx16 = np.zeros((nm_edge, 128, 1024), dtype=np.int16)
    for rr in range(4):
        # call rr list: [src blocks (rr, 0..3) | dst blocks (0..3, rr)],
        # 4096 idxs, wrapped as [16, 256] then replicated to 128 partitions
        lst = np.concatenate(
            [srcv[:, rr].reshape(nm_edge, 2048),
             dstv[:, rr].reshape(nm_edge, 2048)],
            axis=1,
        )  # [nm, 4096]
        wrapped = lst.reshape(nm_edge, 256, 16).transpose(0, 2, 1)  # [nm,16,256]
        idx16[:, :, rr * 256:(rr + 1) * 256] = np.tile(wrapped, (1, 8, 1))
    return pos, idx16


def prepare(x, edge_index, edge_attr, edge_f, w1, b1, w2, b2):
    """Build the Bass program + per-core input maps. Returns (nc, in_maps, meta)."""
    x = np.asarray(x, dtype=np.float32)
    edge_index = np.asarray(edge_index)
    edge_attr = np.asarray(edge_attr, dtype=np.float32)
    edge_f = np.asarray(edge_f, dtype=np.float32)
    w1 = np.asarray(w1, dtype=np.float32)
    b1 = np.asarray(b1, dtype=np.float32)
    w2 = np.asarray(w2, dtype=np.float32)
    b2 = np.asarray(b2, dtype=np.float32)

    n_nodes = x.shape[0]
    n_edges = edge_index.shape[1]
    e_pc = math.ceil(n_edges / N_CORES)
    n_pad = math.ceil(n_nodes / NODE_MACRO) * NODE_MACRO

    b1_nz = bool(np.any(b1))
    b2_nz = bool(np.any(b2))

    ei = edge_index.astype(np.int64)
    cores = []
    nm_edge = 1
    for c in range(N_CORES):
        lo = c * e_pc
        hi = min(lo + e_pc, n_edges)
        src = ei[0, lo:hi]
        dst = ei[1, lo:hi]
        key = (src & 3) * 4 + (dst & 3)
        counts = np.bincount(key, minlength=16)
        nm_edge = max(nm_edge, int(math.ceil(counts.max() / BLK)))
        cores.append((lo, hi, src, dst))

    nc = _build_nc(n_pad, nm_edge, b1_nz, b2_nz)
    e_slots = nm_edge * MACRO

    x_pad = x if n_pad == n_nodes else np.concatenate(
        [x, np.zeros((n_pad - n_nodes, NF), np.float32)], axis=0
    )
    w1t_rep = np.ascontiguousarray(np.tile(w1.T, (2, 1)))          # [128, NH]
    w2t = np.ascontiguousarray(w2.T)                               # [CF, OD]
    b1r = np.ascontiguousarray(np.tile(b1, 8)[None, :])            # [1, 512]
    b2r = np.ascontiguousarray(np.tile(b2, (128, 4)))              # [128, 256]

    in_maps = []
    positions = []
    for c in range(N_CORES):
        lo, hi, src, dst = cores[c]
        pos, idx16 = _shard_core(src, dst, nm_edge)
        positions.append(pos)
        ea_c = np.zeros((e_slots, EA), np.float32)
        ea_c[pos] = edge_attr[lo:hi]
        ef_c = np.zeros((e_slots, EF), np.float32)
        ef_c[pos] = edge_f[lo:hi]
        m = {
            "x": x_pad,
            "w1t": w1t_rep,
            "w2t": w2t,
            "idx": idx16,
            "ea": ea_c,
            "ef": ef_c,
        }
        if b1_nz:
            m["b1r"] = b1r
        if b2_nz:
            m["b2r"] = b2r
        in_maps.append(m)

    meta = {"e_pc": e_pc, "n_edges": n_edges, "positions": positions}
    return nc, in_maps, meta


def kernel(x, edge_index, edge_attr, edge_f, w1, b1, w2, b2):
    global LAST_RESULTS
    nc, in_maps, meta = prepare(
        x, edge_index, edge_attr, edge_f, w1, b1, w2, b2
    )
    res = run_bass_kernel_spmd(nc, in_maps, list(range(N_CORES)), trace=TRACE)
    LAST_RESULTS = res

    e_pc, n_edges = meta["e_pc"], meta["n_edges"]
    parts = []
    for c in range(N_CORES):
        parts.append(res.results[c]["out"][meta["positions"][c]])
    return np.ascontiguousarray(np.concatenate(parts, axis=0), dtype=np.float32)



# revision 5
# speedup vs baseline: 1.2307x; 1.2307x over previous
"""EdgeConv-style GNN message passing kernel for Trainium2 (Bass/Tile).

Computes, for each edge e = (s, d):
    proj = x @ w1.T + b1                      # [N, H]  (node projection)
    h_e  = relu(proj[s] + proj[d])            # [E, H]
    out_e = [h_e | edge_attr_e | edge_f_e] @ w2.T + b2   # [E, O]

Sharding: edges are split evenly across 8 NeuronCores; x and the small
linear weights are replicated.  Each core computes the full proj table
locally, stores it in a DRAM scratch buffer, then gathers the two
endpoint rows per edge with the batched SWDGE gather (InstDMAGatherAnt).

v2 changes vs the f32 baseline:
  - x / edge_attr / edge_f / out and all post-gather compute are fp16
    (rel tolerance is 2e-2; fp16 keeps us ~1e-3).  proj + gather stay
    f32 so every 256B gather descriptor is 100% useful payload.
  - every linear DMA (x load, proj store, ea/ef load, out store) is now
    fully contiguous per partition: the host pre-wraps inputs into
    [*, 128, ...] device layout and unwraps the output.  The proj DRAM
    scratch is stored in a permuted row order rho(n) so the store AP is
    contiguous; the host computes gather indices in rho space.

That gather takes int16 indices (max 32767), so proj rows are addressed
with a stride-4 trick: gather pass r reads rows at base offset r rows
with row stride 4 rows (1024B), index = rho>>2 (<= 25087).  Edges are
bucketed on the host by (rho(src)&3, rho(dst)&3) into 16 blocks of 512
slots per 8192-slot macro; the host permutes edge_attr/edge_f into that
slot order and inverse-permutes the output rows during unshard.

g_all row layout per macro, in units of 128 rows (gathered 16384 rows):
    [sr0(16u) | ds0(16u) | sr1 | ds1 | sr2 | ds2 | sr3 | ds3]
  - gather call r writes units [r*32, (r+1)*32)
  - src rows of block (r,s) at unit  r*32 + s*4      (4 units = 512 rows)
  - dst rows of block (r,s) at unit  s*32 + 16 + r*4
  - hs (edge slot) unit of block (r,s) = r*16 + s*4
"""

import math

import numpy as np

import concourse.bacc as bacc
import concourse.bass as bass
import concourse.mybir as mybir
from concourse import library_config
from concourse.bass_utils import run_bass_kernel_spmd
from concourse.masks import make_identity
from concourse.tile import TileContext, add_dep_helper

F32 = mybir.dt.float32
F16 = mybir.dt.float16
I16 = mybir.dt.int16
RELU = mybir.ActivationFunctionType.Relu

N_CORES = 8
NF = 64   # node feature dim (lin1 input)
NH = 64   # hidden dim (lin1 output)
EA = 16   # edge_attr dim
EF = 16   # edge_f dim
CF = NH + EA + EF  # concat feature dim = 96
OD = 64   # output dim

NODE_MACRO = 1024         # nodes per phase-1 macro tile (8 blocks of 128)
BLK = 512                 # edges per (r,s) bucket block
MACRO = 16 * BLK          # 8192 edge slots per phase-2 macro
N_GROUPS = MACRO // 512   # 16 groups of 512 edge slots per macro

TRACE = False
LAST_RESULTS = None


def _build_nc(
    n_pad: int, nm_edge: int, b1_nz: bool, b2_nz: bool, p2_only: bool = False
) -> bass.Bass:
    assert n_pad % NODE_MACRO == 0
    nm_node = n_pad // NODE_MACRO

    nc = bacc.Bacc()
    x = nc.declare_dram_parameter("x", [nm_node, 128, 8, NF], F16, isOutput=False)
    w1t = nc.declare_dram_parameter("w1t", [128, NH], F16, isOutput=False)
    w2t = nc.declare_dram_parameter("w2t", [CF, OD], F16, isOutput=False)
    # per macro: 4 gather calls x 4096 int16 idx, each wrapped [128, 256]
    idx = nc.declare_dram_parameter("idx", [nm_edge, 128, 1024], I16, isOutput=False)
    ea = nc.declare_dram_parameter("ea", [nm_edge, 128, 64, EA], F16, isOutput=False)
    ef = nc.declare_dram_parameter("ef", [nm_edge, 128, 64, EF], F16, isOutput=False)
    if b1_nz:
        b1r = nc.declare_dram_parameter("b1r", [1, 512], F16, isOutput=False)
    if b2_nz:
        b2r = nc.declare_dram_parameter("b2r", [128, 256], F32, isOutput=False)
    out = nc.declare_dram_parameter("out", [nm_edge, 128, 64, OD], F16, isOutput=True)
    if p2_only:
        proj = nc.declare_dram_parameter("proj", [n_pad, NH], F32, isOutput=False)
    else:
        proj = nc.dram_tensor("proj", [n_pad, NH], F32)
    # rows of `proj` are in rho order: rho = (n & ~1023) + 8*(n%128) + (n%1024)//128
    proj4 = proj[:, :].rearrange("(q r) f -> q r f", r=4)

    with TileContext(nc) as tc:
        with tc.tile_pool(name="const", bufs=1) as cpool:
            libload = nc.gpsimd.load_library(library_config.mlp)
            ident = cpool.tile([128, 128], F16)
            make_identity(nc, ident[:])
            w1t_sb = cpool.tile([128, NH], F16)
            nc.sync.dma_start(out=w1t_sb[:], in_=w1t[:])
            w2t_sb = cpool.tile([CF, OD], F16)
            nc.sync.dma_start(out=w2t_sb[:], in_=w2t[:])
            if b1_nz:
                b1r_sb = cpool.tile([1, 512], F16)
                nc.sync.dma_start(out=b1r_sb[:], in_=b1r[:])
                ones_sb = cpool.tile([1, 128], F16)
                nc.gpsimd.memset(ones_sb[:], 1.0)
            if b2_nz:
                b2r_sb = cpool.tile([128, 256], F32)
                nc.sync.dma_start(out=b2r_sb[:], in_=b2r[:])

            # ---------------- phase 1: proj = x @ w1.T (+ b1) ----------------
            proj_stores = []
            with (
                tc.tile_pool(name="p1", bufs=3) as p1pool,
                tc.tile_pool(name="p1psA", bufs=2, space="PSUM") as ps_xt_pool,
                tc.tile_pool(name="p1psB", bufs=2, space="PSUM") as ps_pr_pool,
            ):
                for m in range(0 if p2_only else nm_node):
                    x_t = p1pool.tile([128, 8, NF], F16, tag="x")
                    nc.sync.dma_start(out=x_t[:], in_=x[m])
                    # single-block transposes: everything stays at SBUF/PSUM
                    # partition 0 (partition-offset matmul operands crash HW)
                    xT_sb = p1pool.tile([64, 1024], F16, tag="xT")
                    for half in range(2):
                        ps_xT = ps_xt_pool.tile([64, 512], F16, tag="psxT")
                        for t4 in range(4):
                            t = half * 4 + t4
                            nc.tensor.transpose(
                                out=ps_xT[:, t4 * 128:(t4 + 1) * 128],
                                in_=x_t[:, t, :],
                                identity=ident[:],
                            )
                        nc.vector.tensor_copy(
                            out=xT_sb[:, half * 512:(half + 1) * 512],
                            in_=ps_xT[:],
                        )
                    ps_proj = ps_pr_pool.tile([128, 512], F32, tag="psproj")
                    if b1_nz:
                        nc.tensor.matmul(
                            out=ps_proj[:],
                            lhsT=ones_sb[:1, :],
                            rhs=b1r_sb[:1, :],
                            start=True,
                            stop=False,
                            skip_group_check=True,
                        )
                    for t in range(8):
                        nc.tensor.matmul(
                            out=ps_proj[:, t * 64:(t + 1) * 64],
                            lhsT=xT_sb[:, t * 128:(t + 1) * 128],
                            rhs=w1t_sb[:64, :],
                            start=not b1_nz,
                            stop=(t == 7) if b1_nz else True,
                            skip_group_check=b1_nz,
                        )
                    proj_sb = p1pool.tile([128, 512], F32, tag="proj")
                    nc.scalar.copy(out=proj_sb[:], in_=ps_proj[:])
                    # rho-ordered store: partition p holds rows
                    # 1024m + 8p + t, contiguous in DRAM per partition
                    st = nc.scalar.dma_start(
                        out=proj[m * 1024:(m + 1) * 1024].rearrange(
                            "(p t) f -> p t f", p=128
                        ),
                        in_=proj_sb[:].rearrange("p (t f) -> p t f", t=8),
                    )
                    proj_stores.append(st)

            join = None
            if not p2_only:
                join = nc.sync.nop(nofuse=True, hint="proj_done_join")
                for st in proj_stores:
                    add_dep_helper(
                        join.ins, st.ins, reason="join waits on proj store"
                    )

            # ---------------- phase 2: per-edge compute ----------------
            with (
                tc.tile_pool(name="p2idx", bufs=2) as idxpool,
                tc.tile_pool(name="p2g", bufs=2) as gpool,
                tc.tile_pool(name="p2hs", bufs=2) as hspool,
                tc.tile_pool(name="p2af", bufs=2) as afpool,
                tc.tile_pool(name="p2ft", bufs=4) as ftpool,
                tc.tile_pool(name="p2o", bufs=3) as opool,
                tc.tile_pool(name="p2psF", bufs=3, space="PSUM") as psf_pool,
                tc.tile_pool(name="p2psO", bufs=3, space="PSUM") as pso_pool,
            ):
                for m in range(nm_edge):
                    idx_t = idxpool.tile([128, 1024], I16, tag="idx")
                    nc.gpsimd.dma_start(out=idx_t[:], in_=idx[m])
                    g = gpool.tile([128, 128, NH], F32, tag="g")
                    for r in range(4):
                        gi = nc.gpsimd.dma_gather(
                            out_ap=g[:, r * 32:(r + 1) * 32, :],
                            in_ap=proj4[:, r, :],
                            idxs_ap=idx_t[:, r * 256:(r + 1) * 256],
                            num_idxs=4096,
                            num_idxs_reg=4096,
                            elem_size=NH,
                            elem_step=4 * NH,
                            single_packet=False,
                        )
                        add_dep_helper(
                            gi.ins, libload.ins, reason="gather after lib load"
                        )
                        if join is not None:
                            add_dep_helper(
                                gi.ins, join.ins, reason="gather waits on proj"
                            )
                    hs = hspool.tile([128, 64, NH], F16, tag="hs")
                    # one add per gather call r: src units (r,0,s,k) pair
                    # with dst units (s,1,r,k); both iterate (s, k, f)
                    g8 = g[:].rearrange(
                        "p (c h s k) f -> p c h s k f", c=4, h=2, s=4
                    )
                    for r in range(4):
                        nc.vector.tensor_add(
                            out=hs[:, r * 16:(r + 1) * 16, :],
                            in0=g8[:, r, 0, :, :, :],
                            in1=g8[:, :, 1, r, :, :],
                        )
                    asm = afpool.tile([128, 64, CF], F16, tag="asm")
                    nc.scalar.activation(
                        out=asm[:, :, 0:NH], in_=hs[:], func=RELU
                    )
                    nc.gpsimd.dma_start(out=asm[:, :, NH:NH + EA], in_=ea[m])
                    nc.gpsimd.dma_start(out=asm[:, :, NH + EA:CF], in_=ef[m])
                    for grp in range(N_GROUPS):
                        ps_f = psf_pool.tile([CF, 512], F16, tag="psf")
                        for j4 in range(4):
                            j = grp * 4 + j4
                            nc.tensor.transpose(
                                out=ps_f[:, j4 * 128:(j4 + 1) * 128],
                                in_=asm[:, j, :],
                                identity=ident[:],
                            )
                        fT = ftpool.tile([CF, 512], F16, tag="ft")
                        if grp % 2 == 0:
                            nc.vector.tensor_copy(out=fT[:], in_=ps_f[:])
                        else:
                            nc.scalar.copy(out=fT[:], in_=ps_f[:])
                        ps_o = pso_pool.tile([128, 256], F32, tag="pso")
                        for j4 in range(4):
                            nc.tensor.matmul(
                                out=ps_o[:, j4 * 64:(j4 + 1) * 64],
                                lhsT=fT[:, j4 * 128:(j4 + 1) * 128],
                                rhs=w2t_sb[:],
                                start=True,
                                stop=True,
                            )
                        if grp % 8 == 0:
                            o_big = opool.tile([128, 8, 256], F16, tag="o")
                        o_sb = o_big[:, grp % 8, :]
                        if b2_nz:
                            nc.vector.tensor_add(
                                out=o_sb, in0=ps_o[:], in1=b2r_sb[:]
                            )
                        elif grp % 2 == 0:
                            nc.scalar.copy(out=o_sb, in_=ps_o[:])
                        else:
                            nc.vector.tensor_copy(out=o_sb, in_=ps_o[:])
                        if grp % 8 == 7:
                            gj0 = (grp - 7) * 4
                            nc.sync.dma_start(
                                out=out[m][:, gj0:gj0 + 32, :],
                                in_=o_big[:].rearrange("p g (j f) -> p (g j) f", j=4),
                            )
    nc.compile()
    return nc


def _rho(n):
    """DRAM row index of node n in the wrapped proj scratch tensor."""
    return (n & ~np.int64(1023)) + 8 * (n & 127) + ((n & 1023) >> 7)


def _shard_core(src, dst, nm_edge):
    """Bucket one core's edges by (rho(src)&3, rho(dst)&3) into the
    macro/block layout.

    Returns (pos, idx16) where pos[e] is the edge's slot index in
    [0, nm_edge*MACRO) and idx16 is the [nm_edge, 128, 1024] int16 gather
    index tensor (indices in rho space).
    """
    n = src.shape[0]
    rs = _rho(src)
    rd = _rho(dst)
    key = ((rs & 3) << 2 | (rd & 3)).astype(np.int8)
    order = np.argsort(key, kind="stable")
    sorted_key = key[order]
    # rank of each sorted element within its bucket
    bstart = np.searchsorted(sorted_key, np.arange(16))
    wb = np.arange(n) - bstart[sorted_key]
    r = (sorted_key >> 2).astype(np.int64)
    s = (sorted_key & 3).astype(np.int64)
    chunk = wb // BLK
    off = wb % BLK
    slot_sorted = chunk * MACRO + (r * 16 + s * 4) * 128 + off
    pos = np.empty(n, dtype=np.int64)
    pos[order] = slot_sorted

    # gather index arrays: SRCV[m, r, s, off] / DSTV[m, s, r, off]
    srcv = np.zeros((nm_edge, 4, 4, BLK), dtype=np.int16)
    dstv = np.zeros((nm_edge, 4, 4, BLK), dtype=np.int16)
    srcq = (rs[order] >> 2).astype(np.int16)
    dstq = (rd[order] >> 2).astype(np.int16)
    srcv[chunk, r, s, off] = srcq
    dstv[chunk, s, r, off] = dstq

    idx16 = np.zeros((nm_edge, 128, 1024), dtype=np.int16)
    for rr in range(4):
        # call rr list: [src blocks (rr, 0..3) | dst blocks (0..3, rr)],
        # 4096 idxs, wrapped as [16, 256] then replicated to 128 partitions
        lst = np.concatenate(
            [srcv[:, rr].reshape(nm_edge, 2048),
             dstv[:, rr].reshape(nm_edge, 2048)],
            axis=1,
        )  # [nm, 4096]
        wrapped = lst.reshape(nm_edge, 256, 16).transpose(0, 2, 1)  # [nm,16,256]
        idx16[:, :, rr * 256:(rr + 1) * 256] = np.tile(wrapped, (1, 8, 1))
    return pos, idx16


def prepare(x, edge_index, edge_attr, edge_f, w1, b1, w2, b2):
    """Build the Bass program + per-core input maps. Returns (nc, in_maps, meta)."""
    x = np.asarray(x, dtype=np.float32)
    edge_index = np.asarray(edge_index)
    edge_attr = np.asarray(edge_attr, dtype=np.float32)
    edge_f = np.asarray(edge_f, dtype=np.float32)
    w1 = np.asarray(w1, dtype=np.float32)
    b1 = np.asarray(b1, dtype=np.float32)
    w2 = np.asarray(w2, dtype=np.float32)
    b2 = np.asarray(b2, dtype=np.float32)

    n_nodes = x.shape[0]
    n_edges = edge_index.shape[1]
    e_pc = math.ceil(n_edges / N_CORES)
    n_pad = math.ceil(n_nodes / NODE_MACRO) * NODE_MACRO
    nm_node = n_pad // NODE_MACRO

    b1_nz = bool(np.any(b1))
    b2_nz = bool(np.any(b2))

    ei = edge_index.astype(np.int64)
    cores = []
    nm_edge = 1
    for c in range(N_CORES):
        lo = c * e_pc
        hi = min(lo + e_pc, n_edges)
        src = ei[0, lo:hi]
        dst = ei[1, lo:hi]
        key = (_rho(src) & 3) * 4 + (_rho(dst) & 3)
        counts = np.bincount(key, minlength=16)
        nm_edge = max(nm_edge, int(math.ceil(counts.max() / BLK)))
        cores.append((lo, hi, src, dst))

    nc = _build_nc(n_pad, nm_edge, b1_nz, b2_nz)
    e_slots = nm_edge * MACRO

    x_pad = x if n_pad == n_nodes else np.concatenate(
        [x, np.zeros((n_pad - n_nodes, NF), np.float32)], axis=0
    )
    # node 1024m + 128t + p  ->  x_w[m, p, t]
    x_w = np.ascontiguousarray(
        x_pad.reshape(nm_node, 8, 128, NF).transpose(0, 2, 1, 3)
    ).astype(np.float16)
    w1t_rep = np.ascontiguousarray(np.tile(w1.T, (2, 1))).astype(np.float16)
    w2t = np.ascontiguousarray(w2.T).astype(np.float16)            # [CF, OD]
    b1r = np.ascontiguousarray(np.tile(b1, 8)[None, :]).astype(np.float16)
    b2r = np.ascontiguousarray(np.tile(b2, (128, 4)))              # [128, 256] f32

    in_maps = []
    positions = []
    for c in range(N_CORES):
        lo, hi, src, dst = cores[c]
        pos, idx16 = _shard_core(src, dst, nm_edge)
        positions.append(pos)
        ea_c = np.zeros((e_slots, EA), np.float32)
        ea_c[pos] = edge_attr[lo:hi]
        ef_c = np.zeros((e_slots, EF), np.float32)
        ef_c[pos] = edge_f[lo:hi]
        # slot m*8192 + 128j + p  ->  [m, p, j]
        ea_w = np.ascontiguousarray(
            ea_c.reshape(nm_edge, 64, 128, EA).transpose(0, 2, 1, 3)
        ).astype(np.float16)
        ef_w = np.ascontiguousarray(
            ef_c.reshape(nm_edge, 64, 128, EF).transpose(0, 2, 1, 3)
        ).astype(np.float16)
        m = {
            "x": x_w,
            "w1t": w1t_rep,
            "w2t": w2t,
            "idx": idx16,
            "ea": ea_w,
            "ef": ef_w,
        }
        if b1_nz:
            m["b1r"] = b1r
        if b2_nz:
            m["b2r"] = b2r
        in_maps.append(m)

    meta = {"e_pc": e_pc, "n_edges": n_edges, "positions": positions}
    return nc, in_maps, meta


def kernel(x, edge_index, edge_attr, edge_f, w1, b1, w2, b2):
    global LAST_RESULTS
    nc, in_maps, meta = prepare(
        x, edge_index, edge_attr, edge_f, w1, b1, w2, b2
    )
    res = run_bass_kernel_spmd(nc, in_maps, list(range(N_CORES)), trace=TRACE)
    LAST_RESULTS = res

    e_pc, n_edges = meta["e_pc"], meta["n_edges"]
    parts = []
    for c in range(N_CORES):
        ow = res.results[c]["out"]          # [nm, 128, 64, OD] fp16
        nm = ow.shape[0]
        # [m, p, gj=4G+j, f] -> slot m*8192 + 512G + 128j + p
        flat = ow.reshape(nm, 128, 16, 4, OD).transpose(0, 2, 3, 1, 4)
        flat = flat.reshape(nm * MACRO, OD).astype(np.float32)
        parts.append(flat[meta["positions"][c]])
    return np.ascontiguousarray(np.concatenate(parts, axis=0), dtype=np.float32)
